# revision 1
# baseline (speedup 1.0000x reference)
"""Trainium2 Bass kernel for CombinedTemporalFocalBCELoss.

Math (exact rewrite of the reference):
  u = x*(2t-1); pt = sigmoid(u); bce = -ln(pt); q = 1-pt
  combined = 0.5*bce + 0.5*(-0.25)*q^2*ln(pt+eps)  ~= 0.125*(q^2+4)*bce
  weight = 1 - 0.2*m,  m = t AND any(t[i-5:i] == 1)
  out = mean(combined * weight)

Device computes, per core (chunk of N/8 elements, data-parallel with a
5-element targets halo):
  C_stored = (q^2 + 4) * ln(pt)            (= -8 * combined)
  acc1[p, tile] = sum_free C_stored        (fused STT accumulation)
  acc2          = sum m * C_stored         (PE ones-matmul reduction)
Host: mean = sum_cores -0.125*(sum acc1 - 0.2*sum acc2) / N
"""

import numpy as np

N_TOTAL = 16_777_216
N_CORES = 8
CHUNK = N_TOTAL // N_CORES      # 2_097_152
P = 128
F = 2048
NT = CHUNK // (P * F)           # 8
HALO = 5

_cache = {}


def _build_nc(reps=1):
    import concourse.bacc as bacc
    import concourse.mybir as mybir
    from concourse.tile import TileContext
    from concourse.ap import AP

    f32 = mybir.dt.float32
    bf16 = mybir.dt.bfloat16
    i32 = mybir.dt.int32
    AF = mybir.ActivationFunctionType
    Alu = mybir.AluOpType

    nc = bacc.Bacc("TRN2", target_bir_lowering=False, debug=False,
                   num_devices=N_CORES)

    x_in = nc.dram_tensor("x", [CHUNK], f32, kind="ExternalInput")
    ext_in = nc.dram_tensor("ext", [CHUNK + HALO], i32, kind="ExternalInput")
    o_acc1 = nc.dram_tensor("acc1", [P, NT], f32, kind="ExternalOutput").ap()
    o_acc2 = nc.dram_tensor("acc2", [1, 512], f32, kind="ExternalOutput").ap()

    x_view = x_in.ap().rearrange("(n p f) -> n p f", p=P, f=F)

    with TileContext(nc) as tc:
        with (
            tc.tile_pool(name="io", bufs=3) as io,
            tc.tile_pool(name="work", bufs=2) as work,
            tc.tile_pool(name="mpool", bufs=NT) as mpool,
            tc.tile_pool(name="rpool", bufs=NT) as rpool,
            tc.tile_pool(name="accp", bufs=1) as accp,
            tc.tile_pool(name="psum", bufs=1, space="PSUM") as psump,
        ):
            ones = accp.tile([P, 1], bf16, tag="ones")
            nc.vector.memset(ones[:], 1.0)
            acc1 = accp.tile([P, NT], f32, tag="acc1")
            psum = psump.tile([1, 512], f32)

            for rep in range(reps):
                m_tiles = []
                r_tiles = []
                # phase 1: mask path + h + sigmoid (sigmoid table set)
                for i in range(NT):
                    xb = io.tile([P, F], bf16, tag="x")
                    nc.gpsimd.dma_start(out=xb[:], in_=x_view[i])
                    e = io.tile([P, F + HALO], bf16, tag="e")
                    nc.gpsimd.dma_start(
                        out=e[:],
                        in_=AP(ext_in, i * P * F, [[F, P], [1, F + HALO]]))

                    A = work.tile([P, F + 3], bf16, tag="A")
                    nc.gpsimd.tensor_add(out=A[:], in0=e[:, 0:F + 3],
                                         in1=e[:, 1:F + 4])
                    B = work.tile([P, F + 1], bf16, tag="B")
                    nc.gpsimd.tensor_add(out=B[:], in0=A[:, 0:F + 1],
                                         in1=A[:, 2:F + 3])
                    w = work.tile([P, F], bf16, tag="w")
                    nc.vector.tensor_add(out=w[:], in0=B[:, 0:F],
                                         in1=e[:, 4:F + 4])

                    m = mpool.tile([P, F], bf16, tag="m")
                    nc.vector.scalar_tensor_tensor(
                        out=m[:], in0=w[:], scalar=1.0,
                        in1=e[:, HALO:F + HALO],
                        op0=Alu.min, op1=Alu.mult)
                    m_tiles.append(m)

                    h = work.tile([P, F], bf16, tag="h")
                    nc.vector.scalar_tensor_tensor(
                        out=h[:], in0=e[:, HALO:F + HALO], scalar=0.5,
                        in1=xb[:], op0=Alu.subtract, op1=Alu.mult)

                    r = rpool.tile([P, F], bf16, tag="r")
                    nc.scalar.activation(r[:], h[:], AF.Sigmoid, scale=2.0)
                    r_tiles.append(r)

                # phase 2: square+ln (natural_log set), C, m*C, PE reduce
                for i in range(NT):
                    q2 = work.tile([P, F], bf16, tag="q2")
                    nc.scalar.activation(q2[:], r_tiles[i][:], AF.Square,
                                         scale=-1.0, bias=1.0)
                    nb = work.tile([P, F], bf16, tag="nb")
                    nc.scalar.activation(nb[:], r_tiles[i][:], AF.Ln)

                    C = work.tile([P, F], bf16, tag="C")
                    nc.vector.scalar_tensor_tensor(
                        out=C[:], in0=q2[:], scalar=4.0, in1=nb[:],
                        op0=Alu.add, op1=Alu.mult,
                        accum_out=acc1[:, i:i + 1])

                    mC = work.tile([P, F], bf16, tag="mC")
                    nc.vector.tensor_mul(out=mC[:], in0=m_tiles[i][:],
                                         in1=C[:])

                    for j in range(F // 512):
                        nc.tensor.matmul(
                            out=psum[0:1, :],
                            lhsT=ones[:, 0:1],
                            rhs=mC[:, j * 512:(j + 1) * 512],
                            start=(i == 0 and j == 0),
                            stop=(i == NT - 1 and j == F // 512 - 1),
                        )

            acc2_sb = accp.tile([1, 512], f32, tag="acc2sb")
            nc.vector.tensor_copy(out=acc2_sb[:], in_=psum[0:1, :])
            nc.sync.dma_start(out=o_acc1, in_=acc1[:])
            nc.sync.dma_start(out=o_acc2, in_=acc2_sb[:])

    nc.compile()
    return nc


def _get_nc(reps=1):
    key = ("nc", reps)
    if key not in _cache:
        _cache[key] = _build_nc(reps)
    return _cache[key]


def _make_in_maps(outputs, targets):
    in_maps = []
    for c in range(N_CORES):
        lo, hi = c * CHUNK, (c + 1) * CHUNK
        halo = (np.zeros(HALO, np.int32) if c == 0
                else targets[lo - HALO:lo])
        ext = np.concatenate([halo, targets[lo:hi]]).astype(np.int32)
        in_maps.append({
            "x": np.ascontiguousarray(outputs[lo:hi], dtype=np.float32),
            "ext": ext,
        })
    return in_maps


def _combine(results):
    total = 0.0
    for res in results:
        a1 = np.asarray(res["acc1"], np.float64).sum()
        a2 = np.asarray(res["acc2"], np.float64).sum()
        total += -0.125 * (a1 - 0.2 * a2)
    return np.float32(total / N_TOTAL)


def kernel(outputs: np.ndarray, targets: np.ndarray) -> np.ndarray:
    from concourse.bass_utils import run_bass_kernel_spmd

    outputs = np.asarray(outputs)
    targets = np.asarray(targets)
    nc = _get_nc()
    res = run_bass_kernel_spmd(nc, _make_in_maps(outputs, targets),
                               core_ids=list(range(N_CORES)))
    return _combine(res.results)


def time_device(outputs, targets, reps=21, iters=3):
    """Estimate per-invocation device time via the wall-clock delta between
    a reps=K build and the reps=1 build (axon RPC overhead cancels)."""
    import time as _time
    from concourse.bass_utils import run_bass_kernel_spmd

    in_maps = _make_in_maps(np.asarray(outputs), np.asarray(targets))

    def best(nc):
        ts = []
        for _ in range(iters):
            t0 = _time.perf_counter()
            run_bass_kernel_spmd(nc, in_maps, core_ids=list(range(N_CORES)))
            ts.append(_time.perf_counter() - t0)
        return min(ts)

    nc1 = _get_nc(1)
    ncK = _get_nc(reps)
    t1 = best(nc1)
    tK = best(ncK)
    dt_ns = (tK - t1) / (reps - 1) * 1e9
    return dt_ns, t1, tK



# revision 4
# speedup vs baseline: 124.9202x; 124.9202x over previous
"""Trainium2 Bass kernel for CombinedTemporalFocalBCELoss.

Math (exact rewrite of the reference):
  u = x*(2t-1); pt = sigmoid(u); bce = -ln(pt); q = 1-pt = sigmoid(-u)
  combined = 0.5*bce + 0.5*0.25*q^2*(-ln(pt)) = -0.125*(q^2+4)*ln(pt)
  weight = 1 - 0.2*m,  m = t AND any(t[i-5:i] == 1)
  out = mean(combined * weight)

Wire format: host ships x as bf16 and targets as e_w = 1 - t/4
(exactly representable: {0.75, 1.0}), so all DMAs are plain HWDGE
copies.  On device (per core, CHUNK = N/8 elements, with an 8-element
targets halo):
  h' = (e_w - 0.875) * x        = -u/8          (DVE stt)
  q  = sigmoid(8*h')            = sigmoid(-u)   (ACT, sigmoid table)
  nb = ln(1 - q)                = ln(pt)        (ACT, natural_log table)
  C  = (q*q + 4) * nb                           (DVE, accum -> acc1 col)
  wmin = min of prev-5 e_w (shift-min tree: gpsimd + DVE)
  m' = max(e_w, wmin)   in {0.75, 1}; weight = 0.2 + 0.8*m'
  mC' = m' * C  -> PE ones-matmul reduction into PSUM (acc2)
Host: loss = -0.125 * sum(0.2*acc1 + 0.8*acc2) / N

The timing loop (`reps`) is a hardware For_i loop, so program size (and
host-side per-call tracing cost) is independent of reps.
"""

import numpy as np
import ml_dtypes

N_TOTAL = 16_777_216
N_CORES = 8
CHUNK = N_TOTAL // N_CORES      # 2_097_152
P = 128
F = 2048
NT = CHUNK // (P * F)           # 8
HPAD = 8                        # halo pad (5 used, 8 for alignment)

_cache = {}


def _build_nc(reps=1):
    import concourse.bacc as bacc
    import concourse.mybir as mybir
    from concourse.tile import TileContext
    from concourse.ap import AP

    f32 = mybir.dt.float32
    bf16 = mybir.dt.bfloat16
    AF = mybir.ActivationFunctionType
    Alu = mybir.AluOpType

    nc = bacc.Bacc("TRN2", target_bir_lowering=False, debug=False,
                   num_devices=N_CORES)

    x_in = nc.dram_tensor("x", [CHUNK], bf16, kind="ExternalInput")
    ew_in = nc.dram_tensor("ew", [CHUNK + HPAD], bf16, kind="ExternalInput")
    o_acc1 = nc.dram_tensor("acc1", [P, NT], f32, kind="ExternalOutput").ap()
    o_acc2 = nc.dram_tensor("acc2", [1, 512], f32, kind="ExternalOutput").ap()

    x_view = x_in.ap().rearrange("(n p f) -> n p f", p=P, f=F)

    with TileContext(nc) as tc:
        with (
            tc.tile_pool(name="io", bufs=3) as io,
            tc.tile_pool(name="wk", bufs=2) as wk,
            tc.tile_pool(name="qpool", bufs=NT) as qpool,
            tc.tile_pool(name="mpool", bufs=NT) as mpool,
            tc.tile_pool(name="accp", bufs=1) as accp,
            tc.tile_pool(name="psum", bufs=1, space="PSUM") as psump,
        ):
            ones = accp.tile([P, 1], bf16, tag="ones")
            nc.vector.memset(ones[:], 1.0)
            acc1 = accp.tile([P, NT], f32, tag="acc1")
            psum = psump.tile([1, 512], f32)

            with tc.For_i(0, reps, 1):
                m_tiles = []
                q_tiles = []
                # phase 1: loads, window min-tree, weight, h', sigmoid
                for i in range(NT):
                    xb = io.tile([P, F], bf16, tag="x")
                    nc.sync.dma_start(out=xb[:], in_=x_view[i])
                    e = io.tile([P, F + HPAD], bf16, tag="e")
                    nc.scalar.dma_start(
                        out=e[:, 0:HPAD],
                        in_=AP(ew_in, i * P * F, [[F, P], [1, HPAD]]))
                    nc.sync.dma_start(
                        out=e[:, HPAD:F + HPAD],
                        in_=AP(ew_in, i * P * F + HPAD, [[F, P], [1, F]]))

                    A = wk.tile([P, F + 6], bf16, tag="A")
                    nc.vector.tensor_tensor(out=A[:], in0=e[:, 0:F + 6],
                                            in1=e[:, 1:F + 7], op=Alu.min)
                    B = wk.tile([P, F + 4], bf16, tag="B")
                    nc.vector.tensor_tensor(out=B[:], in0=A[:, 0:F + 4],
                                            in1=A[:, 2:F + 6], op=Alu.min)
                    wmin = wk.tile([P, F], bf16, tag="wmin")
                    nc.vector.tensor_tensor(out=wmin[:], in0=B[:, 3:F + 3],
                                            in1=e[:, 7:F + 7], op=Alu.min)
                    m = mpool.tile([P, F], bf16, tag="m")
                    nc.vector.tensor_tensor(out=m[:], in0=wmin[:],
                                            in1=e[:, HPAD:F + HPAD],
                                            op=Alu.max)
                    m_tiles.append(m)

                    # h' = (e_w - 0.875) * x = -u/8; split so the subtract
                    # runs on gpsimd (tensor_scalar) and the mult is a DVE
                    # 2x-mode tensor_tensor.
                    tmp = wk.tile([P, F], bf16, tag="tmp")
                    nc.gpsimd.tensor_scalar(
                        out=tmp[:], in0=e[:, HPAD:F + HPAD],
                        scalar1=0.875, scalar2=0.0,
                        op0=Alu.subtract, op1=Alu.add)
                    h = wk.tile([P, F], bf16, tag="h")
                    nc.vector.tensor_mul(out=h[:], in0=tmp[:], in1=xb[:])

                    q = qpool.tile([P, F], bf16, tag="q")
                    nc.scalar.activation(q[:], h[:], AF.Sigmoid, scale=8.0)
                    q_tiles.append(q)

                # phase 2: ln, C, m'*C, PE reduce
                for i in range(NT):
                    q = q_tiles[i]
                    nb = wk.tile([P, F], bf16, tag="nb")
                    nc.scalar.activation(nb[:], q[:], AF.Ln,
                                         scale=-1.0, bias=1.0)
                    q2 = wk.tile([P, F], bf16, tag="q2")
                    nc.vector.tensor_mul(out=q2[:], in0=q[:], in1=q[:])
                    C = wk.tile([P, F], bf16, tag="C")
                    nc.vector.scalar_tensor_tensor(
                        out=C[:], in0=q2[:], scalar=4.0, in1=nb[:],
                        op0=Alu.add, op1=Alu.mult,
                        accum_out=acc1[:, i:i + 1])
                    mC = wk.tile([P, F], bf16, tag="mC")
                    nc.vector.tensor_mul(out=mC[:], in0=m_tiles[i][:],
                                         in1=C[:])
                    for j in range(F // 512):
                        nc.tensor.matmul(
                            out=psum[0:1, :],
                            lhsT=ones[:, 0:1],
                            rhs=mC[:, j * 512:(j + 1) * 512],
                            start=(i == 0 and j == 0),
                            stop=(i == NT - 1 and j == F // 512 - 1),
                        )

            acc2_sb = accp.tile([1, 512], f32, tag="acc2sb")
            nc.vector.tensor_copy(out=acc2_sb[:], in_=psum[0:1, :])
            nc.sync.dma_start(out=o_acc1, in_=acc1[:])
            nc.sync.dma_start(out=o_acc2, in_=acc2_sb[:])

    nc.compile()
    return nc


def _get_nc(reps=1):
    key = ("nc", reps)
    if key not in _cache:
        _cache[key] = _build_nc(reps)
    return _cache[key]


def _make_in_maps(outputs, targets):
    bf = ml_dtypes.bfloat16
    x16 = np.asarray(outputs, dtype=np.float32).astype(bf)
    ew = (1.0 - 0.25 * np.asarray(targets, dtype=np.float32)).astype(bf)
    in_maps = []
    for c in range(N_CORES):
        lo, hi = c * CHUNK, (c + 1) * CHUNK
        halo = np.ones(HPAD, bf)
        if c > 0:
            halo[HPAD - 5:] = ew[lo - 5:lo]
        ext = np.concatenate([halo, ew[lo:hi]])
        in_maps.append({
            "x": np.ascontiguousarray(x16[lo:hi]),
            "ew": np.ascontiguousarray(ext),
        })
    return in_maps


def _combine(results):
    total = 0.0
    for res in results:
        a1 = np.asarray(res["acc1"], np.float64).sum()
        aw = np.asarray(res["acc2"], np.float64).sum()
        total += -0.125 * (0.2 * a1 + 0.8 * aw)
    return np.float32(total / N_TOTAL)


def kernel(outputs: np.ndarray, targets: np.ndarray) -> np.ndarray:
    from concourse.bass_utils import run_bass_kernel_spmd

    outputs = np.asarray(outputs)
    targets = np.asarray(targets)
    nc = _get_nc()
    res = run_bass_kernel_spmd(nc, _make_in_maps(outputs, targets),
                               core_ids=list(range(N_CORES)))
    return _combine(res.results)


class _Runner:
    """Persistent-jit SPMD runner (mirrors run_bass_via_pjrt but reuses the
    jitted callable and device-resident inputs, so repeated calls measure
    dispatch + device execution only)."""

    def __init__(self, nc, in_maps):
        import jax
        import concourse.mybir as mybir
        from concourse import bass2jax
        from concourse.bass2jax import _bass_exec_p, install_neuronx_cc_hook
        from jax.sharding import Mesh, PartitionSpec, NamedSharding

        install_neuronx_cc_hook()
        self.jax = jax
        n_cores = len(in_maps)
        in_names, out_names, out_avals, zero_outs = [], [], [], []
        for alloc in nc.m.functions[0].allocations:
            if not isinstance(alloc, mybir.MemoryLocationSet):
                continue
            name = alloc.memorylocations[0].name
            if alloc.kind == "ExternalInput":
                in_names.append(name)
            elif alloc.kind == "ExternalOutput":
                out_names.append(name)
                shape = tuple(alloc.tensor_shape)
                dtype = mybir.dt.np(alloc.dtype)
                out_avals.append(jax.core.ShapedArray(shape, dtype))
                zero_outs.append(np.zeros(shape, dtype))
        partition_name = (nc.partition_id_tensor.name
                          if nc.partition_id_tensor else None)
        if partition_name is not None:
            in_names = [n for n in in_names if n != partition_name]
        n_params = len(in_names)
        all_in_names = in_names + out_names
        if partition_name is not None:
            all_in_names.append(partition_name)

        def _body(*args):
            operands = list(args)
            if partition_name is not None:
                operands.append(bass2jax.partition_id_tensor())
            return tuple(_bass_exec_p.bind(
                *operands,
                out_avals=tuple(out_avals),
                in_names=tuple(all_in_names),
                out_names=tuple(out_names),
                lowering_input_output_aliases=(),
                sim_require_finite=True,
                sim_require_nnan=True,
                nc=nc,
            ))

        devices = jax.devices()[:n_cores]
        mesh = Mesh(np.asarray(devices), ("core",))
        in_specs = (PartitionSpec("core"),) * (n_params + len(out_names))
        out_specs = (PartitionSpec("core"),) * len(out_names)
        donate = tuple(range(n_params, n_params + len(out_names)))
        self.fn = jax.jit(
            jax.shard_map(_body, mesh=mesh, in_specs=in_specs,
                          out_specs=out_specs, check_vma=False),
            donate_argnums=donate, keep_unused=True)
        self.sh = NamedSharding(mesh, PartitionSpec("core"))
        self.dev_in = [
            jax.device_put(np.concatenate(
                [np.asarray(in_maps[c][nm]) for c in range(n_cores)],
                axis=0), self.sh)
            for nm in in_names]
        self.zero_outs = [
            np.zeros((n_cores * z.shape[0], *z.shape[1:]), z.dtype)
            for z in zero_outs]

    def run(self):
        import time as _time
        jax = self.jax
        zouts = [jax.device_put(z, self.sh) for z in self.zero_outs]
        for z in zouts:
            z.block_until_ready()
        t0 = _time.perf_counter()
        outs = self.fn(*self.dev_in, *zouts)
        for o in outs:
            o.block_until_ready()
        return _time.perf_counter() - t0

    def time_min(self, iters, warmup=2):
        for _ in range(warmup):
            self.run()
        return min(self.run() for _ in range(iters))


def time_device(outputs, targets, reps=401, iters=8):
    """Per-invocation device time: wall-clock delta between a reps=K and a
    reps=1 hardware-loop build (identical program size, so host dispatch
    overhead cancels), divided by K-1."""
    in_maps = _make_in_maps(np.asarray(outputs), np.asarray(targets))
    r1 = _Runner(_get_nc(1), in_maps)
    rK = _Runner(_get_nc(reps), in_maps)
    t1 = r1.time_min(iters)
    tK = rK.time_min(iters)
    dt_ns = (tK - t1) / (reps - 1) * 1e9
    return dt_ns, t1, tK
